# revision 23
# baseline (speedup 1.0000x reference)
"""Trainium2 Bass kernel for a MiniGPT block:
out = causal_softmax((h Wq^T + bq)(h Wk^T + bk)^T) (h Wv^T + bv),  h = tok_emb[x] + pos_emb

Sharding: data-parallel over batch (B=8) across 8 NeuronCores, one batch row per
core; weights/embeddings replicated. No collectives needed.

Per-core pipeline (all activations/weights bf16 on-chip, fp32 PSUM accumulation;
set DTYPE="f32r" for a ~2e-4 rel-err variant at ~30% more time):
  A) constant loads ordered by need; token rows gathered by id via 16 indirect
     DMAs (128 rows each)
  B) h tiles PE-transposed into H^T [e, t] layout; the pos_emb^T add is fused
     into the PSUM->SBUF eviction
  C) Q^T = Wq H^T, K^T = Wk H^T (weights stationary), V = H Wv^T (H^T
     stationary); biases fused into the evictions; a ones-column appended to V
     makes the PV matmul emit softmax denominators for free
  D) attention in groups of 4 query tiles: S^T blocks (K^T tile stationary,
     512-wide Q^T chunks moving) -> exp on ScalarE (scores are tiny: no max
     subtraction needed; masked positions zeroed by a 0/1 triangular mask after
     exp) -> P^T V' accumulated in PSUM with the PV stage lagged two steps
     behind S^T/exp so the PE never stalls -> normalize by the ones-column
     rowsum at eviction.
  The next group's transposes/QKV are emitted interleaved into the current
  group's attention loop to keep the PE dense.
"""

import numpy as np

B = 8
DTYPE = "bf16"  # "f32r" or "bf16"
T = 2048
E = 256
V = 50257
P = 128
NT = T // P  # 16 token tiles
EC = E // P  # 2 embedding chunks
G = 4        # query tiles per group
NG = NT // G

_cache = {}


def _build_nc():
    import concourse.bacc as bacc
    import concourse.bass as bass
    import concourse.mybir as mybir
    import concourse.tile as tile

    f32 = mybir.dt.float32
    f32r = mybir.dt.float32r if DTYPE == "f32r" else mybir.dt.bfloat16
    i32 = mybir.dt.int32
    Exp = mybir.ActivationFunctionType.Exp

    nc = bacc.Bacc("TRN2", target_bir_lowering=False, debug=False)

    xi = nc.dram_tensor("xi", [P, NT], i32, kind="ExternalInput")
    temb = nc.dram_tensor("temb", [V, E], f32r, kind="ExternalInput")
    posT = nc.dram_tensor("posT", [P, EC, T], f32r, kind="ExternalInput")
    wqT = nc.dram_tensor("wqT", [P, EC, E], f32r, kind="ExternalInput")
    wkT = nc.dram_tensor("wkT", [P, EC, E], f32r, kind="ExternalInput")
    wvT = nc.dram_tensor("wvT", [P, EC, E], f32r, kind="ExternalInput")
    bqc = nc.dram_tensor("bqc", [P, EC], f32, kind="ExternalInput")
    bkc = nc.dram_tensor("bkc", [P, EC], f32, kind="ExternalInput")
    bvr = nc.dram_tensor("bvr", [1, E], f32, kind="ExternalInput")
    ident = nc.dram_tensor("ident", [P, P], f32r, kind="ExternalInput")
    trim = nc.dram_tensor("trim", [P, P], f32r, kind="ExternalInput")
    onec = nc.dram_tensor("onec", [P, NT, 2], f32r, kind="ExternalInput")
    out = nc.dram_tensor("out", [T, E], f32, kind="ExternalOutput")

    with tile.TileContext(nc) as tc:
        with (
            tc.tile_pool(name="const", bufs=1) as cp,
            tc.tile_pool(name="acts", bufs=1) as ap,
            tc.tile_pool(name="work", bufs=3) as wp,
            tc.tile_pool(name="outp", bufs=3) as op,
            tc.tile_pool(name="psum", bufs=1, space="PSUM") as psp,
        ):
            # ---- Phase A: loads, ordered so early-needed tiles land first ----
            x_sb = cp.tile([P, NT], i32)
            nc.sync.dma_start(x_sb[:], xi[:])
            id_sb = cp.tile([P, P], f32r, tag="ident")
            nc.sync.dma_start(id_sb[:], ident[:])
            posT_sb = cp.tile([P, EC, T], f32r)
            for c in range(EC):
                nc.sync.dma_start(posT_sb[:, c, :], posT[:, c, :])
            tri_sb = cp.tile([P, P], f32r, tag="tri")
            nc.sync.dma_start(tri_sb[:], trim[:])
            bq_sb = cp.tile([P, EC], f32, tag="bq")
            nc.sync.dma_start(bq_sb[:, :], bqc[:, :])
            bk_sb = cp.tile([P, EC], f32, tag="bk")
            nc.sync.dma_start(bk_sb[:, :], bkc[:, :])
            bv_sb = cp.tile([P, E], f32, tag="bv")
            nc.sync.dma_start(bv_sb[:, :], bvr[:, :].to_broadcast([P, E]))

            # persistent activations
            ht_sb = ap.tile([P, EC, T], f32r, tag="ht")
            qt_sb = ap.tile([P, EC, T], f32r, tag="qt")
            kt_sb = ap.tile([P, EC, T], f32r, tag="kt")
            v_sb = ap.tile([P, NT, E + 2], f32r, tag="v")
            nc.sync.dma_start(v_sb[:, :, E : E + 2], onec[:, :, :])

            wq_sb = cp.tile([P, EC, E], f32r, tag="wq")
            nc.sync.dma_start(wq_sb[:, :, :], wqT[:, :, :])
            wk_sb = cp.tile([P, EC, E], f32r, tag="wk")
            nc.sync.dma_start(wk_sb[:, :, :], wkT[:, :, :])
            wv_sb = cp.tile([P, EC, E], f32r, tag="wv")
            nc.sync.dma_start(wv_sb[:, :, :], wvT[:, :, :])

            # ---- gathers: all emitted up front; they self-pace on the Q7 queue ----
            h_tiles = []
            for i in range(NT):
                hti = wp.tile([P, E], f32r, tag=f"h{i}", bufs=1, name=f"hti{i}")
                nc.gpsimd.indirect_dma_start(
                    out=hti[:],
                    out_offset=None,
                    in_=temb[:, :],
                    in_offset=bass.IndirectOffsetOnAxis(ap=x_sb[:, i : i + 1], axis=0),
                )
                h_tiles.append(hti)

            # ---- wavefront: per query-tile group, with the next group's
            # prep (transposes + QKV) interleaved into this group's attention ----
            def emit_transpose(i, c):
                pst = psp.tile([P, P], f32r, tag="misc", bufs=2, name="pst")
                nc.tensor.transpose(
                    pst[:], h_tiles[i][:, c * P : (c + 1) * P], id_sb[:]
                )
                nc.vector.tensor_add(
                    ht_sb[:, c, i * P : (i + 1) * P],
                    pst[:],
                    posT_sb[:, c, i * P : (i + 1) * P],
                )

            def emit_qk(g, proj, fc):
                wsb, bsb, dst = (
                    (wq_sb, bq_sb, qt_sb) if proj == 0 else (wk_sb, bk_sb, kt_sb)
                )
                ps = psp.tile([P, 512], f32, tag="misc", bufs=2, name="ps_qk")
                for c in range(EC):
                    nc.tensor.matmul(
                        ps[:],
                        lhsT=wsb[:, c, fc * P : (fc + 1) * P],
                        rhs=ht_sb[:, c, g * 512 : (g + 1) * 512],
                        start=(c == 0),
                        stop=(c == EC - 1),
                    )
                nc.vector.tensor_scalar_add(
                    dst[:, fc, g * 512 : (g + 1) * 512], ps[:], bsb[:, fc : fc + 1]
                )

            def emit_v(i):
                psv = psp.tile([P, E], f32, tag="misc", bufs=2, name="ps_v")
                for c in range(EC):
                    nc.tensor.matmul(
                        psv[:],
                        lhsT=ht_sb[:, c, i * P : (i + 1) * P],
                        rhs=wv_sb[:, c, :],
                        start=(c == 0),
                        stop=(c == EC - 1),
                    )
                nc.vector.tensor_add(v_sb[:, i, 0:E], psv[:], bv_sb[:, :])

            def prep_steps(g):
                steps = []
                for i in range(G * g, G * g + G):
                    for c in range(EC):
                        steps.append(lambda i=i, c=c: emit_transpose(i, c))
                for proj in range(2):
                    for fc in range(EC):
                        steps.append(lambda g=g, p=proj, fc=fc: emit_qk(g, p, fc))
                for i in range(G * g, G * g + G):
                    steps.append(lambda i=i: emit_v(i))
                return steps

            for step in prep_steps(0):
                step()

            for g in range(NG):
                pending = prep_steps(g + 1) if g + 1 < NG else []

                o_ps = [
                    psp.tile([P, E + 2], f32, tag=f"o{ii}", bufs=1, name=f"o_ps{ii}")
                    for ii in range(G)
                ]

                def emit_evict(ii, g=g, o_ps=o_ps):
                    i = G * g + ii
                    rec = wp.tile([P, 1], f32, tag="rec")
                    nc.vector.reciprocal(rec[:], o_ps[ii][:, E : E + 1])
                    o_sb = op.tile([P, E], f32, tag="osb")
                    nc.vector.tensor_scalar_mul(o_sb[:], o_ps[ii][:, 0:E], rec[:, 0:1])
                    nc.sync.dma_start(out[i * P : (i + 1) * P, :], o_sb[:])

                def emit_pv(j, pt, g=g, o_ps=o_ps):
                    jj = j - G * g
                    iis = list(range(max(0, jj), G))
                    if jj >= 0:
                        iis = iis[1:] + iis[:1]  # diagonal PV last (waits on mask)
                    for ii in iis:
                        i = G * g + ii
                        nc.tensor.matmul(
                            o_ps[ii][:],
                            lhsT=pt[:, ii * P : (ii + 1) * P],
                            rhs=v_sb[:, j, :],
                            start=(j == 0),
                            stop=(j == i),
                        )
                        if j == i:
                            emit_evict(ii)

                njs = G * g + G
                pipeline = []
                for j in range(njs):
                    jj = j - G * g
                    moff = max(0, jj) * P          # diagonal block position
                    soff = (
                        moff if DTYPE == "bf16" else min(max(0, jj), 2) * P
                    )  # fp32r needs moving dim >= 256
                    s_ps = psp.tile([P, 512], f32, tag="s", bufs=2, name="s_ps")
                    for c in range(EC):
                        nc.tensor.matmul(
                            s_ps[:, soff:512],
                            lhsT=kt_sb[:, c, j * P : (j + 1) * P],
                            rhs=qt_sb[:, c, g * 512 + soff : (g + 1) * 512],
                            start=(c == 0),
                            stop=(c == EC - 1),
                        )
                    pt = wp.tile([P, 512], f32r, tag="pt", bufs=4)
                    nc.scalar.activation(pt[:, soff:512], s_ps[:, soff:512], Exp)
                    if jj >= 0:
                        nc.vector.tensor_mul(
                            pt[:, moff : moff + P], pt[:, moff : moff + P], tri_sb[:]
                        )
                    pipeline.append((j, pt))
                    if len(pipeline) > 3:
                        emit_pv(*pipeline.pop(0))
                    # sprinkle next-group prep to keep PE dense
                    total = len(prep_steps(g + 1)) if g + 1 < NG else 0
                    while pending and (j + 1) * total // njs > total - len(pending):
                        pending.pop(0)()
                for item in pipeline:
                    emit_pv(*item)
                while pending:
                    pending.pop(0)()


    nc.compile()
    return nc


def _get_nc():
    if "nc" not in _cache:
        _cache["nc"] = _build_nc()
    return _cache["nc"]


def _np_dt():
    if DTYPE == "f32r":
        return np.float32
    import ml_dtypes

    return ml_dtypes.bfloat16


def _prep_inputs(x, tok_emb, pos_emb, Wq, bq, Wk, bk, Wv, bv):
    ndt = _np_dt()
    x = np.asarray(x).astype(np.int32)
    tok_emb = np.ascontiguousarray(np.asarray(tok_emb, dtype=np.float32).astype(ndt))
    pos_emb = np.asarray(pos_emb, dtype=np.float32)

    def w_arr(w):
        # [P, EC, E]: w_arr[p, c, f] = W[f, c*128+p]
        return np.ascontiguousarray(
            np.asarray(w, dtype=np.float32)
            .T.reshape(EC, P, E)
            .transpose(1, 0, 2)
            .astype(ndt)
        )

    def b_arr(b):
        return np.ascontiguousarray(
            np.asarray(b, dtype=np.float32).reshape(EC, P).T
        )

    posT = np.ascontiguousarray(
        pos_emb.T.reshape(EC, P, T).transpose(1, 0, 2).astype(ndt)
    )  # posT[p, c, t] = pos_emb[t, c*128+p]
    common = {
        "temb": tok_emb,
        "posT": posT,
        "wqT": w_arr(Wq),
        "wkT": w_arr(Wk),
        "wvT": w_arr(Wv),
        "bqc": b_arr(bq),
        "bkc": b_arr(bk),
        "bvr": np.asarray(bv, dtype=np.float32).reshape(1, E),
        "ident": np.eye(P, dtype=np.float32).astype(ndt),
        "trim": (np.arange(P)[:, None] <= np.arange(P)[None, :]).astype(ndt),
        "onec": np.broadcast_to(
            np.array([1.0, 0.0], dtype=np.float32).astype(ndt), (P, NT, 2)
        ).copy(),
    }
    in_maps = []
    for b_i in range(B):
        xi = np.ascontiguousarray(x[b_i].reshape(NT, P).T)  # xi[p, i] = x[b, i*128+p]
        in_maps.append({**common, "xi": xi})
    return in_maps


def _run(inputs, trace=False):
    from concourse.bass_utils import run_bass_kernel_spmd

    if trace:
        # the axon NTFF-profile hook is not pre-registered in this image
        try:
            import sys as _sys
            import types as _types

            import antenv as _antenv

            if "antenv.axon_hooks" not in _sys.modules:
                _holder = [None]
                _mod = _types.ModuleType("antenv.axon_hooks")
                _mod.set_axon_ntff_profile_hook = lambda h: _holder.__setitem__(0, h)
                _mod.get_axon_ntff_profile_hook = lambda: _holder[0]
                _sys.modules["antenv.axon_hooks"] = _mod
                _antenv.axon_hooks = _mod
                from trn_agent_boot.trn_boot import _ntff_profile_via_ctypes

                _mod.set_axon_ntff_profile_hook(
                    _ntff_profile_via_ctypes("/opt/axon/libaxon_pjrt.so")
                )
        except Exception:
            trace = False

    nc = _get_nc()
    in_maps = _prep_inputs(**inputs)
    res = run_bass_kernel_spmd(
        nc, in_maps, core_ids=list(range(B)), trace=trace
    )
    outs = np.stack([res.results[b]["out"] for b in range(B)], axis=0)
    return outs, res


def kernel(**inputs):
    outs, _ = _run(inputs, trace=False)
    return outs


# revision 25
# speedup vs baseline: 1.0074x; 1.0074x over previous
"""Trainium2 Bass kernel for a MiniGPT block:
out = causal_softmax((h Wq^T + bq)(h Wk^T + bk)^T) (h Wv^T + bv),  h = tok_emb[x] + pos_emb

Sharding: data-parallel over batch (B=8) across 8 NeuronCores, one batch row per
core; weights/embeddings replicated. No collectives needed.

Per-core pipeline (all activations/weights bf16 on-chip, fp32 PSUM accumulation;
set DTYPE="f32r" for a ~2e-4 rel-err variant at ~30% more time):
  A) constant loads ordered by need; token rows gathered by id via 16 indirect
     DMAs (128 rows each)
  B) h tiles PE-transposed into H^T [e, t] layout; the pos_emb^T add is fused
     into the PSUM->SBUF eviction
  C) Q^T = Wq H^T, K^T = Wk H^T (weights stationary), V = H Wv^T (H^T
     stationary); biases fused into the evictions; a ones-column appended to V
     makes the PV matmul emit softmax denominators for free
  D) attention in groups of 4 query tiles: S^T blocks (K^T tile stationary,
     512-wide Q^T chunks moving) -> exp on ScalarE (scores are tiny: no max
     subtraction needed; masked positions zeroed by a 0/1 triangular mask after
     exp) -> P^T V' accumulated in PSUM with the PV stage lagged two steps
     behind S^T/exp so the PE never stalls -> normalize by the ones-column
     rowsum at eviction.
  The next group's transposes/QKV are emitted interleaved into the current
  group's attention loop to keep the PE dense.
"""

import numpy as np

B = 8
DTYPE = "bf16"  # "f32r" or "bf16"
T = 2048
E = 256
V = 50257
P = 128
NT = T // P  # 16 token tiles
EC = E // P  # 2 embedding chunks
G = 4        # query tiles per group
NG = NT // G

_cache = {}


def _build_nc(fused):
    import concourse.bacc as bacc
    import concourse.bass as bass
    import concourse.mybir as mybir
    import concourse.tile as tile

    f32 = mybir.dt.float32
    f32r = mybir.dt.float32r if DTYPE == "f32r" else mybir.dt.bfloat16
    i32 = mybir.dt.int32
    Exp = mybir.ActivationFunctionType.Exp

    nc = bacc.Bacc("TRN2", target_bir_lowering=False, debug=False)

    xi = nc.dram_tensor("xi", [P, NT], i32, kind="ExternalInput")
    temb = nc.dram_tensor("temb", [V, E], f32r, kind="ExternalInput")
    posT = nc.dram_tensor("posT", [P, EC, T], f32r, kind="ExternalInput")
    if fused:
        # biases are zero: S = H (Wq^T Wk) H^T. Natural-layout Wq/Wk feed a tiny
        # on-device D = Wk^T Wq; K^T projection disappears entirely.
        wqT = nc.dram_tensor("wqn", [P, EC, E], f32r, kind="ExternalInput")
        wkT = nc.dram_tensor("wkn", [P, EC, E], f32r, kind="ExternalInput")
    else:
        wqT = nc.dram_tensor("wqT", [P, EC, E], f32r, kind="ExternalInput")
        wkT = nc.dram_tensor("wkT", [P, EC, E], f32r, kind="ExternalInput")
    wvT = nc.dram_tensor("wvT", [P, EC, E], f32r, kind="ExternalInput")
    bqc = nc.dram_tensor("bqc", [P, EC], f32, kind="ExternalInput")
    bkc = nc.dram_tensor("bkc", [P, EC], f32, kind="ExternalInput")
    bvr = nc.dram_tensor("bvr", [1, E], f32, kind="ExternalInput")
    ident = nc.dram_tensor("ident", [P, P], f32r, kind="ExternalInput")
    trim = nc.dram_tensor("trim", [P, P], f32r, kind="ExternalInput")
    onec = nc.dram_tensor("onec", [P, NT, 2], f32r, kind="ExternalInput")
    out = nc.dram_tensor("out", [T, E], f32, kind="ExternalOutput")

    with tile.TileContext(nc) as tc:
        with (
            tc.tile_pool(name="const", bufs=1) as cp,
            tc.tile_pool(name="acts", bufs=1) as ap,
            tc.tile_pool(name="work", bufs=3) as wp,
            tc.tile_pool(name="outp", bufs=3) as op,
            tc.tile_pool(name="psum", bufs=1, space="PSUM") as psp,
        ):
            # ---- Phase A: loads, ordered so early-needed tiles land first ----
            x_sb = cp.tile([P, NT], i32)
            nc.sync.dma_start(x_sb[:], xi[:])
            id_sb = cp.tile([P, P], f32r, tag="ident")
            nc.sync.dma_start(id_sb[:], ident[:])
            posT_sb = cp.tile([P, EC, T], f32r)
            for c in range(EC):
                nc.sync.dma_start(posT_sb[:, c, :], posT[:, c, :])
            tri_sb = cp.tile([P, P], f32r, tag="tri")
            nc.sync.dma_start(tri_sb[:], trim[:])
            bq_sb = cp.tile([P, EC], f32, tag="bq")
            nc.sync.dma_start(bq_sb[:, :], bqc[:, :])
            bk_sb = cp.tile([P, EC], f32, tag="bk")
            nc.sync.dma_start(bk_sb[:, :], bkc[:, :])
            bv_sb = cp.tile([P, E], f32, tag="bv")
            nc.sync.dma_start(bv_sb[:, :], bvr[:, :].to_broadcast([P, E]))

            # persistent activations
            ht_sb = ap.tile([P, EC, T], f32r, tag="ht")
            qt_sb = ap.tile([P, EC, T], f32r, tag="qt")
            kt_sb = None if fused else ap.tile([P, EC, T], f32r, tag="kt", name="kt_sb")
            d_sb = cp.tile([P, EC, E], f32r, tag="dmat", name="d_sb") if fused else None
            v_sb = ap.tile([P, NT, E + 2], f32r, tag="v")
            nc.sync.dma_start(v_sb[:, :, E : E + 2], onec[:, :, :])

            wq_sb = cp.tile([P, EC, E], f32r, tag="wq")
            nc.sync.dma_start(wq_sb[:, :, :], wqT[:, :, :])
            wk_sb = cp.tile([P, EC, E], f32r, tag="wk")
            nc.sync.dma_start(wk_sb[:, :, :], wkT[:, :, :])
            wv_sb = cp.tile([P, EC, E], f32r, tag="wv")
            nc.sync.dma_start(wv_sb[:, :, :], wvT[:, :, :])

            if fused:
                # D[e1, e] = sum_f Wk[f, e1] Wq[f, e]
                for m in range(EC):
                    psd = psp.tile([P, E], f32, tag="misc", bufs=2, name="psd")
                    for c in range(EC):
                        nc.tensor.matmul(
                            psd[:],
                            lhsT=wk_sb[:, c, m * P : (m + 1) * P],
                            rhs=wq_sb[:, c, :],
                            start=(c == 0),
                            stop=(c == EC - 1),
                        )
                    nc.vector.tensor_copy(d_sb[:, m, :], psd[:])

            # ---- gathers: all emitted up front; they self-pace on the Q7 queue ----
            h_tiles = []
            for i in range(NT):
                hti = wp.tile([P, E], f32r, tag=f"h{i}", bufs=1, name=f"hti{i}")
                nc.gpsimd.indirect_dma_start(
                    out=hti[:],
                    out_offset=None,
                    in_=temb[:, :],
                    in_offset=bass.IndirectOffsetOnAxis(ap=x_sb[:, i : i + 1], axis=0),
                )
                h_tiles.append(hti)

            # ---- wavefront: per query-tile group, with the next group's
            # prep (transposes + QKV) interleaved into this group's attention ----
            def emit_transpose(i, c):
                pst = psp.tile([P, P], f32r, tag="misc", bufs=2, name="pst")
                nc.tensor.transpose(
                    pst[:], h_tiles[i][:, c * P : (c + 1) * P], id_sb[:]
                )
                nc.vector.tensor_add(
                    ht_sb[:, c, i * P : (i + 1) * P],
                    pst[:],
                    posT_sb[:, c, i * P : (i + 1) * P],
                )

            def emit_qk(g, proj, fc):
                if fused:
                    wsb, bsb, dst = d_sb, bq_sb, qt_sb
                else:
                    wsb, bsb, dst = (
                        (wq_sb, bq_sb, qt_sb) if proj == 0 else (wk_sb, bk_sb, kt_sb)
                    )
                ps = psp.tile([P, 512], f32, tag="misc", bufs=2, name="ps_qk")
                for c in range(EC):
                    nc.tensor.matmul(
                        ps[:],
                        lhsT=wsb[:, c, fc * P : (fc + 1) * P],
                        rhs=ht_sb[:, c, g * 512 : (g + 1) * 512],
                        start=(c == 0),
                        stop=(c == EC - 1),
                    )
                nc.vector.tensor_scalar_add(
                    dst[:, fc, g * 512 : (g + 1) * 512], ps[:], bsb[:, fc : fc + 1]
                )

            def emit_v(i):
                psv = psp.tile([P, E], f32, tag="misc", bufs=2, name="ps_v")
                for c in range(EC):
                    nc.tensor.matmul(
                        psv[:],
                        lhsT=ht_sb[:, c, i * P : (i + 1) * P],
                        rhs=wv_sb[:, c, :],
                        start=(c == 0),
                        stop=(c == EC - 1),
                    )
                nc.vector.tensor_add(v_sb[:, i, 0:E], psv[:], bv_sb[:, :])

            def prep_steps(g):
                steps = []
                for i in range(G * g, G * g + G):
                    for c in range(EC):
                        steps.append(lambda i=i, c=c: emit_transpose(i, c))
                for proj in range(1 if fused else 2):
                    for fc in range(EC):
                        steps.append(lambda g=g, p=proj, fc=fc: emit_qk(g, p, fc))
                for i in range(G * g, G * g + G):
                    steps.append(lambda i=i: emit_v(i))
                return steps

            for step in prep_steps(0):
                step()

            for g in range(NG):
                pending = prep_steps(g + 1) if g + 1 < NG else []

                o_ps = [
                    psp.tile([P, E + 2], f32, tag=f"o{ii}", bufs=1, name=f"o_ps{ii}")
                    for ii in range(G)
                ]

                def emit_evict(ii, g=g, o_ps=o_ps):
                    i = G * g + ii
                    rec = wp.tile([P, 1], f32, tag="rec")
                    nc.vector.reciprocal(rec[:], o_ps[ii][:, E : E + 1])
                    o_sb = op.tile([P, E], f32, tag="osb")
                    nc.vector.tensor_scalar_mul(o_sb[:], o_ps[ii][:, 0:E], rec[:, 0:1])
                    nc.sync.dma_start(out[i * P : (i + 1) * P, :], o_sb[:])

                def emit_pv(j, pt, g=g, o_ps=o_ps):
                    jj = j - G * g
                    iis = list(range(max(0, jj), G))
                    if jj >= 0:
                        iis = iis[1:] + iis[:1]  # diagonal PV last (waits on mask)
                    for ii in iis:
                        i = G * g + ii
                        nc.tensor.matmul(
                            o_ps[ii][:],
                            lhsT=pt[:, ii * P : (ii + 1) * P],
                            rhs=v_sb[:, j, :],
                            start=(j == 0),
                            stop=(j == i),
                        )
                        if j == i:
                            emit_evict(ii)

                njs = G * g + G
                pipeline = []
                for j in range(njs):
                    jj = j - G * g
                    moff = max(0, jj) * P          # diagonal block position
                    soff = (
                        moff if DTYPE == "bf16" else min(max(0, jj), 2) * P
                    )  # fp32r needs moving dim >= 256
                    s_ps = psp.tile([P, 512], f32, tag="s", bufs=2, name="s_ps")
                    for c in range(EC):
                        st_lhs = ht_sb if fused else kt_sb
                        nc.tensor.matmul(
                            s_ps[:, soff:512],
                            lhsT=st_lhs[:, c, j * P : (j + 1) * P],
                            rhs=qt_sb[:, c, g * 512 + soff : (g + 1) * 512],
                            start=(c == 0),
                            stop=(c == EC - 1),
                        )
                    pt = wp.tile([P, 512], f32r, tag="pt", bufs=4)
                    nc.scalar.activation(pt[:, soff:512], s_ps[:, soff:512], Exp)
                    if jj >= 0:
                        nc.vector.tensor_mul(
                            pt[:, moff : moff + P], pt[:, moff : moff + P], tri_sb[:]
                        )
                    pipeline.append((j, pt))
                    if len(pipeline) > 3:
                        emit_pv(*pipeline.pop(0))
                    # sprinkle next-group prep to keep PE dense
                    total = len(prep_steps(g + 1)) if g + 1 < NG else 0
                    while pending and (j + 1) * total // njs > total - len(pending):
                        pending.pop(0)()
                for item in pipeline:
                    emit_pv(*item)
                while pending:
                    pending.pop(0)()


    nc.compile()
    return nc


def _get_nc(fused):
    key = ("nc", fused)
    if key not in _cache:
        _cache[key] = _build_nc(fused)
    return _cache[key]


def _np_dt():
    if DTYPE == "f32r":
        return np.float32
    import ml_dtypes

    return ml_dtypes.bfloat16


def _prep_inputs(x, tok_emb, pos_emb, Wq, bq, Wk, bk, Wv, bv, fused):
    ndt = _np_dt()
    x = np.asarray(x).astype(np.int32)
    tok_emb = np.ascontiguousarray(np.asarray(tok_emb, dtype=np.float32).astype(ndt))
    pos_emb = np.asarray(pos_emb, dtype=np.float32)

    def w_arr(w):
        # [P, EC, E]: w_arr[p, c, f] = W[f, c*128+p]
        return np.ascontiguousarray(
            np.asarray(w, dtype=np.float32)
            .T.reshape(EC, P, E)
            .transpose(1, 0, 2)
            .astype(ndt)
        )

    def b_arr(b):
        return np.ascontiguousarray(
            np.asarray(b, dtype=np.float32).reshape(EC, P).T
        )

    posT = np.ascontiguousarray(
        pos_emb.T.reshape(EC, P, T).transpose(1, 0, 2).astype(ndt)
    )  # posT[p, c, t] = pos_emb[t, c*128+p]
    def w_nat(w):
        # [P, EC, E]: w_nat[p, c, e] = W[c*128+p, e]
        return np.ascontiguousarray(
            np.asarray(w, dtype=np.float32).reshape(EC, P, E).transpose(1, 0, 2).astype(ndt)
        )

    common = {
        "temb": tok_emb,
        "posT": posT,
        **(
            {"wqn": w_nat(Wq), "wkn": w_nat(Wk)}
            if fused
            else {"wqT": w_arr(Wq), "wkT": w_arr(Wk)}
        ),
        "wvT": w_arr(Wv),
        "bqc": b_arr(bq),
        "bkc": b_arr(bk),
        "bvr": np.asarray(bv, dtype=np.float32).reshape(1, E),
        "ident": np.eye(P, dtype=np.float32).astype(ndt),
        "trim": (np.arange(P)[:, None] <= np.arange(P)[None, :]).astype(ndt),
        "onec": np.broadcast_to(
            np.array([1.0, 0.0], dtype=np.float32).astype(ndt), (P, NT, 2)
        ).copy(),
    }
    in_maps = []
    for b_i in range(B):
        xi = np.ascontiguousarray(x[b_i].reshape(NT, P).T)  # xi[p, i] = x[b, i*128+p]
        in_maps.append({**common, "xi": xi})
    return in_maps


def _run(inputs, trace=False):
    from concourse.bass_utils import run_bass_kernel_spmd

    if trace:
        # the axon NTFF-profile hook is not pre-registered in this image
        try:
            import sys as _sys
            import types as _types

            import antenv as _antenv

            if "antenv.axon_hooks" not in _sys.modules:
                _holder = [None]
                _mod = _types.ModuleType("antenv.axon_hooks")
                _mod.set_axon_ntff_profile_hook = lambda h: _holder.__setitem__(0, h)
                _mod.get_axon_ntff_profile_hook = lambda: _holder[0]
                _sys.modules["antenv.axon_hooks"] = _mod
                _antenv.axon_hooks = _mod
                from trn_agent_boot.trn_boot import _ntff_profile_via_ctypes

                _mod.set_axon_ntff_profile_hook(
                    _ntff_profile_via_ctypes("/opt/axon/libaxon_pjrt.so")
                )
        except Exception:
            trace = False

    fused = not (np.any(np.asarray(inputs["bq"])) or np.any(np.asarray(inputs["bk"])))
    nc = _get_nc(fused)
    in_maps = _prep_inputs(**inputs, fused=fused)
    res = run_bass_kernel_spmd(
        nc, in_maps, core_ids=list(range(B)), trace=trace
    )
    outs = np.stack([res.results[b]["out"] for b in range(B)], axis=0)
    return outs, res


def kernel(**inputs):
    outs, _ = _run(inputs, trace=False)
    return outs


# revision 26
# speedup vs baseline: 1.0396x; 1.0320x over previous
"""Trainium2 Bass kernel for a MiniGPT block:
out = causal_softmax((h Wq^T + bq)(h Wk^T + bk)^T) (h Wv^T + bv),  h = tok_emb[x] + pos_emb

Sharding: data-parallel over batch (B=8) across 8 NeuronCores, one batch row per
core; weights/embeddings replicated. No collectives needed.

Per-core pipeline (all activations/weights bf16 on-chip, fp32 PSUM accumulation;
set DTYPE="f32r" for a ~2e-4 rel-err variant at ~30% more time):
  A) constant loads ordered by need; token rows gathered by id via 16 indirect
     DMAs (128 rows each)
  B) h tiles PE-transposed into H^T [e, t] layout; the pos_emb^T add is fused
     into the PSUM->SBUF eviction
  C) Q^T = Wq H^T, K^T = Wk H^T (weights stationary), V = H Wv^T (H^T
     stationary); biases fused into the evictions; a ones-column appended to V
     makes the PV matmul emit softmax denominators for free
  D) attention in groups of 4 query tiles: S^T blocks (K^T tile stationary,
     512-wide Q^T chunks moving) -> exp on ScalarE (scores are tiny: no max
     subtraction needed; masked positions zeroed by a 0/1 triangular mask after
     exp) -> P^T V' accumulated in PSUM with the PV stage lagged two steps
     behind S^T/exp so the PE never stalls -> normalize by the ones-column
     rowsum at eviction.
  The next group's transposes/QKV are emitted interleaved into the current
  group's attention loop to keep the PE dense.
"""

import numpy as np

B = 8
DTYPE = "bf16"  # "f32r" or "bf16"
T = 2048
E = 256
V = 50257
P = 128
NT = T // P  # 16 token tiles
EC = E // P  # 2 embedding chunks
G = 4        # query tiles per group
NG = NT // G

_cache = {}


def _build_nc(fused):
    import concourse.bacc as bacc
    import concourse.bass as bass
    import concourse.mybir as mybir
    import concourse.tile as tile

    f32 = mybir.dt.float32
    f32r = mybir.dt.float32r if DTYPE == "f32r" else mybir.dt.bfloat16
    i32 = mybir.dt.int32
    Exp = mybir.ActivationFunctionType.Exp

    nc = bacc.Bacc("TRN2", target_bir_lowering=False, debug=False)

    xi = nc.dram_tensor("xi", [P, NT], i32, kind="ExternalInput")
    temb = nc.dram_tensor("temb", [V, E], f32r, kind="ExternalInput")
    posT = nc.dram_tensor("posT", [P, EC, T], f32r, kind="ExternalInput")
    if fused:
        # biases are zero: S = H (Wq^T Wk) H^T. Natural-layout Wq/Wk feed a tiny
        # on-device D = Wk^T Wq; K^T projection disappears entirely.
        wqT = nc.dram_tensor("wqn", [P, EC, E], f32r, kind="ExternalInput")
        wkT = nc.dram_tensor("wkn", [P, EC, E], f32r, kind="ExternalInput")
    else:
        wqT = nc.dram_tensor("wqT", [P, EC, E], f32r, kind="ExternalInput")
        wkT = nc.dram_tensor("wkT", [P, EC, E], f32r, kind="ExternalInput")
    wvT = nc.dram_tensor("wvT", [P, EC, E], f32r, kind="ExternalInput")
    bqc = nc.dram_tensor("bqc", [P, EC], f32, kind="ExternalInput")
    bkc = nc.dram_tensor("bkc", [P, EC], f32, kind="ExternalInput")
    bvr = nc.dram_tensor("bvr", [1, E], f32, kind="ExternalInput")
    ident = nc.dram_tensor("ident", [P, P], f32r, kind="ExternalInput")
    trim = nc.dram_tensor("trim", [P, P], f32r, kind="ExternalInput")
    onec = nc.dram_tensor("onec", [P, NT, 2], f32r, kind="ExternalInput")
    out = nc.dram_tensor("out", [T, E], f32, kind="ExternalOutput")

    with tile.TileContext(nc) as tc:
        with (
            tc.tile_pool(name="const", bufs=1) as cp,
            tc.tile_pool(name="acts", bufs=1) as ap,
            tc.tile_pool(name="work", bufs=3) as wp,
            tc.tile_pool(name="outp", bufs=3) as op,
            tc.tile_pool(name="psum", bufs=1, space="PSUM") as psp,
        ):
            # ---- Phase A: loads, ordered so early-needed tiles land first ----
            x_sb = cp.tile([P, NT], i32)
            nc.sync.dma_start(x_sb[:], xi[:])
            id_sb = cp.tile([P, P], f32r, tag="ident")
            nc.sync.dma_start(id_sb[:], ident[:])
            wq_sb = cp.tile([P, EC, E], f32r, tag="wq")
            nc.sync.dma_start(wq_sb[:, :, :], wqT[:, :, :])
            wk_sb = cp.tile([P, EC, E], f32r, tag="wk")
            nc.sync.dma_start(wk_sb[:, :, :], wkT[:, :, :])
            posT_sb = cp.tile([P, EC, T], f32r)
            for c in range(EC):
                nc.sync.dma_start(posT_sb[:, c, :], posT[:, c, :])
            tri_sb = cp.tile([P, P], f32r, tag="tri")
            nc.sync.dma_start(tri_sb[:], trim[:])
            bq_sb = cp.tile([P, EC], f32, tag="bq")
            nc.sync.dma_start(bq_sb[:, :], bqc[:, :])
            bk_sb = cp.tile([P, EC], f32, tag="bk")
            nc.sync.dma_start(bk_sb[:, :], bkc[:, :])
            bv_sb = cp.tile([P, E], f32, tag="bv")
            nc.sync.dma_start(bv_sb[:, :], bvr[:, :].to_broadcast([P, E]))

            # persistent activations
            ht_sb = ap.tile([P, EC, T], f32r, tag="ht")
            qt_sb = ap.tile([P, EC, T], f32r, tag="qt")
            kt_sb = None if fused else ap.tile([P, EC, T], f32r, tag="kt", name="kt_sb")
            d_sb = cp.tile([P, EC, E], f32r, tag="dmat", name="d_sb") if fused else None
            v_sb = ap.tile([P, NT, E + 2], f32r, tag="v")
            nc.sync.dma_start(v_sb[:, :, E : E + 2], onec[:, :, :])

            wv_sb = cp.tile([P, EC, E], f32r, tag="wv")
            nc.sync.dma_start(wv_sb[:, :, :], wvT[:, :, :])

            # ---- gathers: all emitted up front; they self-pace on the Q7 queue ----
            h_tiles = []
            for i in range(NT):
                hti = wp.tile([P, E], f32r, tag=f"h{i}", bufs=1, name=f"hti{i}")
                nc.gpsimd.indirect_dma_start(
                    out=hti[:],
                    out_offset=None,
                    in_=temb[:, :],
                    in_offset=bass.IndirectOffsetOnAxis(ap=x_sb[:, i : i + 1], axis=0),
                )
                h_tiles.append(hti)

            # ---- wavefront: per query-tile group, with the next group's
            # prep (transposes + QKV) interleaved into this group's attention ----
            def emit_transpose(i, c):
                pst = psp.tile([P, P], f32r, tag="misc", bufs=2, name="pst")
                nc.tensor.transpose(
                    pst[:], h_tiles[i][:, c * P : (c + 1) * P], id_sb[:]
                )
                nc.vector.tensor_add(
                    ht_sb[:, c, i * P : (i + 1) * P],
                    pst[:],
                    posT_sb[:, c, i * P : (i + 1) * P],
                )

            def emit_qk(g, proj, fc):
                if fused:
                    wsb, bsb, dst = d_sb, bq_sb, qt_sb
                else:
                    wsb, bsb, dst = (
                        (wq_sb, bq_sb, qt_sb) if proj == 0 else (wk_sb, bk_sb, kt_sb)
                    )
                ps = psp.tile([P, 512], f32, tag="misc", bufs=2, name="ps_qk")
                for c in range(EC):
                    nc.tensor.matmul(
                        ps[:],
                        lhsT=wsb[:, c, fc * P : (fc + 1) * P],
                        rhs=ht_sb[:, c, g * 512 : (g + 1) * 512],
                        start=(c == 0),
                        stop=(c == EC - 1),
                    )
                nc.vector.tensor_scalar_add(
                    dst[:, fc, g * 512 : (g + 1) * 512], ps[:], bsb[:, fc : fc + 1]
                )

            def emit_v(i):
                psv = psp.tile([P, E], f32, tag="misc", bufs=2, name="ps_v")
                for c in range(EC):
                    nc.tensor.matmul(
                        psv[:],
                        lhsT=ht_sb[:, c, i * P : (i + 1) * P],
                        rhs=wv_sb[:, c, :],
                        start=(c == 0),
                        stop=(c == EC - 1),
                    )
                nc.vector.tensor_add(v_sb[:, i, 0:E], psv[:], bv_sb[:, :])

            def prep_steps(g):
                steps = []
                for i in range(G * g, G * g + G):
                    for c in range(EC):
                        steps.append(lambda i=i, c=c: emit_transpose(i, c))
                for proj in range(1 if fused else 2):
                    for fc in range(EC):
                        steps.append(lambda g=g, p=proj, fc=fc: emit_qk(g, p, fc))
                for i in range(G * g, G * g + G):
                    steps.append(lambda i=i: emit_v(i))
                return steps

            steps0 = prep_steps(0)
            for step in steps0[: G * EC]:  # group-0 transposes first
                step()
            if fused:
                # D[e1, e] = sum_f Wk[f, e1] Wq[f, e]
                for m in range(EC):
                    psd = psp.tile([P, E], f32, tag="misc", bufs=2, name="psd")
                    for c in range(EC):
                        nc.tensor.matmul(
                            psd[:],
                            lhsT=wk_sb[:, c, m * P : (m + 1) * P],
                            rhs=wq_sb[:, c, :],
                            start=(c == 0),
                            stop=(c == EC - 1),
                        )
                    nc.vector.tensor_copy(d_sb[:, m, :], psd[:])
            for step in steps0[G * EC :]:
                step()

            for g in range(NG):
                pending = prep_steps(g + 1) if g + 1 < NG else []

                o_ps = [
                    psp.tile([P, E + 2], f32, tag=f"o{ii}", bufs=1, name=f"o_ps{ii}")
                    for ii in range(G)
                ]

                def emit_evict(ii, g=g, o_ps=o_ps):
                    i = G * g + ii
                    rec = wp.tile([P, 1], f32, tag="rec")
                    nc.vector.reciprocal(rec[:], o_ps[ii][:, E : E + 1])
                    o_sb = op.tile([P, E], f32, tag="osb")
                    nc.vector.tensor_scalar_mul(o_sb[:], o_ps[ii][:, 0:E], rec[:, 0:1])
                    nc.sync.dma_start(out[i * P : (i + 1) * P, :], o_sb[:])

                def emit_pv(j, pt, g=g, o_ps=o_ps):
                    jj = j - G * g
                    iis = list(range(max(0, jj), G))
                    if jj >= 0:
                        iis = iis[1:] + iis[:1]  # diagonal PV last (waits on mask)
                    for ii in iis:
                        i = G * g + ii
                        nc.tensor.matmul(
                            o_ps[ii][:],
                            lhsT=pt[:, ii * P : (ii + 1) * P],
                            rhs=v_sb[:, j, :],
                            start=(j == 0),
                            stop=(j == i),
                        )
                        if j == i:
                            emit_evict(ii)

                njs = G * g + G
                pipeline = []
                for j in range(njs):
                    jj = j - G * g
                    moff = max(0, jj) * P          # diagonal block position
                    soff = (
                        moff if DTYPE == "bf16" else min(max(0, jj), 2) * P
                    )  # fp32r needs moving dim >= 256
                    s_ps = psp.tile([P, 512], f32, tag="s", bufs=2, name="s_ps")
                    for c in range(EC):
                        st_lhs = ht_sb if fused else kt_sb
                        nc.tensor.matmul(
                            s_ps[:, soff:512],
                            lhsT=st_lhs[:, c, j * P : (j + 1) * P],
                            rhs=qt_sb[:, c, g * 512 + soff : (g + 1) * 512],
                            start=(c == 0),
                            stop=(c == EC - 1),
                        )
                    pt = wp.tile([P, 512], f32r, tag="pt", bufs=4)
                    nc.scalar.activation(pt[:, soff:512], s_ps[:, soff:512], Exp)
                    if jj >= 0:
                        nc.vector.tensor_mul(
                            pt[:, moff : moff + P], pt[:, moff : moff + P], tri_sb[:]
                        )
                    pipeline.append((j, pt))
                    if len(pipeline) > 3:
                        emit_pv(*pipeline.pop(0))
                    # sprinkle next-group prep to keep PE dense
                    total = len(prep_steps(g + 1)) if g + 1 < NG else 0
                    while pending and (j + 1) * total // njs > total - len(pending):
                        pending.pop(0)()
                for item in pipeline:
                    emit_pv(*item)
                while pending:
                    pending.pop(0)()


    nc.compile()
    return nc


def _get_nc(fused):
    key = ("nc", fused)
    if key not in _cache:
        _cache[key] = _build_nc(fused)
    return _cache[key]


def _np_dt():
    if DTYPE == "f32r":
        return np.float32
    import ml_dtypes

    return ml_dtypes.bfloat16


def _prep_inputs(x, tok_emb, pos_emb, Wq, bq, Wk, bk, Wv, bv, fused):
    ndt = _np_dt()
    x = np.asarray(x).astype(np.int32)
    tok_emb = np.ascontiguousarray(np.asarray(tok_emb, dtype=np.float32).astype(ndt))
    pos_emb = np.asarray(pos_emb, dtype=np.float32)

    def w_arr(w):
        # [P, EC, E]: w_arr[p, c, f] = W[f, c*128+p]
        return np.ascontiguousarray(
            np.asarray(w, dtype=np.float32)
            .T.reshape(EC, P, E)
            .transpose(1, 0, 2)
            .astype(ndt)
        )

    def b_arr(b):
        return np.ascontiguousarray(
            np.asarray(b, dtype=np.float32).reshape(EC, P).T
        )

    posT = np.ascontiguousarray(
        pos_emb.T.reshape(EC, P, T).transpose(1, 0, 2).astype(ndt)
    )  # posT[p, c, t] = pos_emb[t, c*128+p]
    def w_nat(w):
        # [P, EC, E]: w_nat[p, c, e] = W[c*128+p, e]
        return np.ascontiguousarray(
            np.asarray(w, dtype=np.float32).reshape(EC, P, E).transpose(1, 0, 2).astype(ndt)
        )

    common = {
        "temb": tok_emb,
        "posT": posT,
        **(
            {"wqn": w_nat(Wq), "wkn": w_nat(Wk)}
            if fused
            else {"wqT": w_arr(Wq), "wkT": w_arr(Wk)}
        ),
        "wvT": w_arr(Wv),
        "bqc": b_arr(bq),
        "bkc": b_arr(bk),
        "bvr": np.asarray(bv, dtype=np.float32).reshape(1, E),
        "ident": np.eye(P, dtype=np.float32).astype(ndt),
        "trim": (np.arange(P)[:, None] <= np.arange(P)[None, :]).astype(ndt),
        "onec": np.broadcast_to(
            np.array([1.0, 0.0], dtype=np.float32).astype(ndt), (P, NT, 2)
        ).copy(),
    }
    in_maps = []
    for b_i in range(B):
        xi = np.ascontiguousarray(x[b_i].reshape(NT, P).T)  # xi[p, i] = x[b, i*128+p]
        in_maps.append({**common, "xi": xi})
    return in_maps


def _run(inputs, trace=False):
    from concourse.bass_utils import run_bass_kernel_spmd

    if trace:
        # the axon NTFF-profile hook is not pre-registered in this image
        try:
            import sys as _sys
            import types as _types

            import antenv as _antenv

            if "antenv.axon_hooks" not in _sys.modules:
                _holder = [None]
                _mod = _types.ModuleType("antenv.axon_hooks")
                _mod.set_axon_ntff_profile_hook = lambda h: _holder.__setitem__(0, h)
                _mod.get_axon_ntff_profile_hook = lambda: _holder[0]
                _sys.modules["antenv.axon_hooks"] = _mod
                _antenv.axon_hooks = _mod
                from trn_agent_boot.trn_boot import _ntff_profile_via_ctypes

                _mod.set_axon_ntff_profile_hook(
                    _ntff_profile_via_ctypes("/opt/axon/libaxon_pjrt.so")
                )
        except Exception:
            trace = False

    fused = not (np.any(np.asarray(inputs["bq"])) or np.any(np.asarray(inputs["bk"])))
    nc = _get_nc(fused)
    in_maps = _prep_inputs(**inputs, fused=fused)
    res = run_bass_kernel_spmd(
        nc, in_maps, core_ids=list(range(B)), trace=trace
    )
    outs = np.stack([res.results[b]["out"] for b in range(B)], axis=0)
    return outs, res


def kernel(**inputs):
    outs, _ = _run(inputs, trace=False)
    return outs
